# revision 1
# baseline (speedup 1.0000x reference)
"""Depthwise morphological (max-plus) dilation, 3x3, stride 1, zero-pad 1.

out[b,c,i,j] = max_{p,q} ( x_pad[b,c,i+p,j+q] + se[c,p,q] )

Sharding: pure data parallel over batch (16 batches -> 8 cores x 2).
On-core layout: partition dim = 2 batches x 64 channels = 128 planes;
each partition processes its own plane in row-blocks. The host supplies
x zero-padded to [P, H+2, W+2] and fp16-converted, so all 9 taps are
free-dim-shifted 3D access patterns of one SBUF tile and the device
does no zero-fill.

Engine split: the 9 taps are divided into a DVE max-chain and a GpSimd
max-chain (merged by one final DVE max); the ScalarE (ACT) engine
produces the shifted+biased addends for part of the DVE chain. fp16
tensor ops run in the DVE 2x/4x perf modes; accuracy ~6e-4 rel.
"""

import numpy as np

B, C, H, W = 16, 64, 256, 256
K = 3
NCORES = 8
BPC = B // NCORES          # batches per core
P = BPC * C                # 128 partitions
HP, WP = H + 2, W + 2      # host-padded plane

COMPUTE = "f16"            # "f16" (fast, ~6e-4 rel err) or "f32" (exact)
R = 32                     # output rows per block
# tap index t = di*3+dj for the 3x3 window; assignment below
GP_TAPS = ()              # GpSimd cannot run elementwise in this backend
ACT_TAPS = (1, 2, 4, 6, 7)  # adds done on ScalarE, max on DVE

_prog_cache = {}


def _build(compute=COMPUTE, h=H, r=R, gp_taps=GP_TAPS, act_taps=ACT_TAPS,
           reps=1, xbufs=2, abufs=2, tmpgbufs=1):
    """Build the Bass program for one core: x [P,h+2,W+2] -> o [P,h,W]."""
    import concourse.bacc as bacc
    import concourse.mybir as mybir
    from concourse.tile import TileContext

    add, mx = mybir.AluOpType.add, mybir.AluOpType.max
    dt = mybir.dt.float16 if compute == "f16" else mybir.dt.float32

    nc = bacc.Bacc()
    x_d = nc.dram_tensor("x", [P, h + 2, W + 2], dt, kind="ExternalInput")
    se_d = nc.dram_tensor("se", [P, K * K], mybir.dt.float32, kind="ExternalInput")
    o_d = nc.dram_tensor("o", [P, h, W], dt, kind="ExternalOutput")

    gp_taps = tuple(gp_taps)
    act_taps = tuple(act_taps)
    all_taps = list(range(K * K))
    # DVE-added taps first in the chain, ACT-fed links last: gives the
    # (slower-per-op) ACT engine maximal lead time to stay ahead.
    dve_taps = [t for t in all_taps if t not in gp_taps and t not in act_taps]
    dve_taps += [t for t in all_taps if t in act_taps]
    assert not (set(gp_taps) & set(act_taps))

    with TileContext(nc) as tc:
        with (
            tc.tile_pool(name="cpool", bufs=1) as cpool,
            tc.tile_pool(name="xpool", bufs=xbufs) as xpool,
            tc.tile_pool(name="apool", bufs=abufs) as apool,
            tc.tile_pool(name="tpool", bufs=2) as tpool,
        ):
            se_sb = cpool.tile([P, K * K], mybir.dt.float32)
            nc.sync.dma_start(out=se_sb[:], in_=se_d[:, :])

            for r0 in [v for _ in range(reps) for v in range(0, h, r)]:
                xt = xpool.tile([P, r + 2, W + 2], dt, tag="xt")
                nc.sync.dma_start(out=xt[:], in_=x_d[:, r0 : r0 + r + 2, :])

                acc = apool.tile([P, r, W], dt, tag="acc")

                def src(t):
                    di, dj = divmod(t, K)
                    return xt[:, di : di + r, dj : dj + W]

                def sca(t):
                    return se_sb[:, t : t + 1]

                # GpSimd partial chain (its own accumulator)
                accg = None
                if gp_taps:
                    accg = tpool.tile([P, r, W], dt, tag="accg")
                    nc.gpsimd.tensor_scalar(accg[:], src(gp_taps[0]),
                                            sca(gp_taps[0]), None, add)
                    for t in gp_taps[1:]:
                        tmpg = tpool.tile([P, r, W], dt, tag="tmpg",
                                          bufs=tmpgbufs)
                        nc.gpsimd.tensor_scalar(tmpg[:], src(t), sca(t), None, add)
                        nc.gpsimd.tensor_tensor(accg[:], accg[:], tmpg[:], mx)

                # DVE chain; ACT produces addends for act_taps
                t0 = dve_taps[0]
                nc.vector.tensor_scalar(acc[:], src(t0), sca(t0), None, add)
                for t in dve_taps[1:]:
                    if compute == "f16":
                        if t in act_taps:
                            tmp = tpool.tile([P, r, W], dt, tag="tmp2")
                            nc.scalar.add(tmp[:], src(t), sca(t))
                        else:
                            tmp = tpool.tile([P, r, W], dt, tag="tmp")
                            nc.vector.tensor_scalar(tmp[:], src(t), sca(t), None, add)
                        nc.vector.tensor_tensor(acc[:], acc[:], tmp[:], mx)
                    else:
                        nc.vector.scalar_tensor_tensor(
                            acc[:], src(t), sca(t), acc[:], add, mx)

                if accg is not None:
                    nc.vector.tensor_tensor(acc[:], acc[:], accg[:], mx)

                nc.sync.dma_start(out=o_d[:, r0 : r0 + r, :], in_=acc[:])
    # bacc legalization (splits >1-wait instructions into event semaphores)
    nc.finalize()
    return nc


def _get_prog(key=("default",)):
    if key not in _prog_cache:
        _prog_cache[key] = _build()
    return _prog_cache[key]


def _pad_shard(x_shard, np_dt):
    """[BPC,C,H,W] fp32 -> zero-padded [P, H+2, W+2] in np_dt."""
    xp = np.zeros((P, HP, WP), np_dt)
    xp[:, 1 : H + 1, 1 : W + 1] = x_shard.reshape(P, H, W)
    return xp


def _run(x, se, **spmd_kwargs):
    from concourse.bass_utils import run_bass_kernel_spmd

    nc = _get_prog()
    np_dt = np.float16 if COMPUTE == "f16" else np.float32
    x = np.asarray(x)
    se_p = np.tile(np.asarray(se, np.float32).reshape(C, K * K), (BPC, 1))
    in_maps = [
        {"x": _pad_shard(x[k * BPC : (k + 1) * BPC], np_dt), "se": se_p}
        for k in range(NCORES)
    ]
    res = run_bass_kernel_spmd(nc, in_maps, core_ids=list(range(NCORES)), **spmd_kwargs)
    out = np.empty((B, C, H, W), np.float32)
    for k in range(NCORES):
        out[k * BPC : (k + 1) * BPC] = (
            res.results[k]["o"].astype(np.float32).reshape(BPC, C, H, W)
        )
    return out, res


def kernel(x: np.ndarray, se: np.ndarray) -> np.ndarray:
    return _run(x, se)[0]



# revision 2
# speedup vs baseline: 22.7277x; 22.7277x over previous
"""Depthwise morphological (max-plus) dilation, 3x3, stride 1, zero-pad 1.

out[b,c,i,j] = max_{p,q} ( x_pad[b,c,i+p,j+q] + se[c,p,q] )

Sharding: pure data parallel over batch (16 batches -> 8 cores x 2).
On-core layout: partition dim = 2 batches x 64 channels = 128 planes;
each partition processes its own plane in row-blocks of R=32 output
rows. The host supplies x zero-padded to [P, H+2, W+2] and converted to
fp16, so all 9 taps are free-dim-shifted 3D views of one SBUF tile and
the device does no zero-fill.

Engine schedule (per output element: 9 scalar-adds + 8 tensor-maxes):
  DVE  tensor_scalar add runs in 4x perf mode   (~0.26 ns/elem/part)
  DVE  tensor_tensor max runs in 2x perf mode   (~0.52 ns/elem/part)
  ACT  activation-bias add, 1 elem/cycle @1.2GHz (~0.83 ns/elem/part)
GpSimd/Pool cannot run elementwise ops on this backend (ISA check
rejects TensorTensor on Pool), and the fused scalar_tensor_tensor loses
all DVE perf modes, so the optimum is unfused ops split DVE/ACT:
3 adds + 8 maxes on DVE (~4.95 ns/elem) and 6 adds on ACT (~5.0),
i.e. both engines balanced at the ~325 us/core compute floor
(memory roofline is ~95 us; 17 exact elementwise ops/elem bind first).
fp16 compute gives ~6e-4 max rel err vs the fp32 reference.
"""

import numpy as np

B, C, H, W = 16, 64, 256, 256
K = 3
NCORES = 8
BPC = B // NCORES          # batches per core
P = BPC * C                # 128 partitions
HP, WP = H + 2, W + 2      # host-padded plane

COMPUTE = "f16"            # "f16" (fast, ~6e-4 rel err) or "f32" (exact)
R = 32                     # output rows per block
DVE_TAPS = (0, 5, 8)       # tap t = di*3+dj; DVE does these adds (t0=root)
ACT_TAPS = (1, 2, 3, 4, 6, 7)  # adds on ScalarE; maxes on DVE

_prog_cache = {}


def _build(compute=COMPUTE, h=H, r=R, reps=1,
           dve_taps=DVE_TAPS, act_taps=ACT_TAPS,
           xbufs=2, abufs=2, atmp_bufs=4, dtmp_bufs=2):
    """Build the Bass program for one core: x [P,h+2,W+2] -> o [P,h,W]."""
    import concourse.bacc as bacc
    import concourse.mybir as mybir
    from concourse.tile import TileContext

    add, mx = mybir.AluOpType.add, mybir.AluOpType.max
    dt = mybir.dt.float16 if compute == "f16" else mybir.dt.float32

    nc = bacc.Bacc()
    x_d = nc.dram_tensor("x", [P, h + 2, W + 2], dt, kind="ExternalInput")
    se_d = nc.dram_tensor("se", [P, K * K], mybir.dt.float32, kind="ExternalInput")
    o_d = nc.dram_tensor("o", [P, h, W], dt, kind="ExternalOutput")

    assert len(dve_taps) + len(act_taps) == K * K

    with TileContext(nc) as tc:
        with (
            tc.tile_pool(name="cpool", bufs=1) as cpool,
            tc.tile_pool(name="xpool", bufs=xbufs) as xpool,
            tc.tile_pool(name="apool", bufs=abufs) as apool,
            tc.tile_pool(name="tpool", bufs=2) as tpool,
        ):
            se_sb = cpool.tile([P, K * K], mybir.dt.float32)
            nc.sync.dma_start(out=se_sb[:], in_=se_d[:, :])

            for r0 in [v for _ in range(reps) for v in range(0, h, r)]:
                xt = xpool.tile([P, r + 2, W + 2], dt, tag="xt")
                nc.sync.dma_start(out=xt[:], in_=x_d[:, r0 : r0 + r + 2, :])

                acc = apool.tile([P, r, W], dt, tag="acc")

                def src(t):
                    di, dj = divmod(t, K)
                    return xt[:, di : di + r, dj : dj + W]

                def sca(t):
                    return se_sb[:, t : t + 1]

                # DVE-only links first (independent of ACT), ACT-fed links
                # last: gives the slower-per-op ACT engine lead time.
                t0 = dve_taps[0]
                nc.vector.tensor_scalar(acc[:], src(t0), sca(t0), None, add)
                for t in dve_taps[1:]:
                    tmp = tpool.tile([P, r, W], dt, tag="dtmp", bufs=dtmp_bufs)
                    nc.vector.tensor_scalar(tmp[:], src(t), sca(t), None, add)
                    nc.vector.tensor_tensor(acc[:], acc[:], tmp[:], mx)
                for t in act_taps:
                    tmp = tpool.tile([P, r, W], dt, tag="atmp", bufs=atmp_bufs)
                    nc.scalar.add(tmp[:], src(t), sca(t))
                    nc.vector.tensor_tensor(acc[:], acc[:], tmp[:], mx)

                nc.sync.dma_start(out=o_d[:, r0 : r0 + r, :], in_=acc[:])
    # bacc legalization (splits >1-wait instructions into event semaphores)
    nc.finalize()
    return nc


def _get_prog(key=("default",)):
    if key not in _prog_cache:
        _prog_cache[key] = _build()
    return _prog_cache[key]


def _pad_shard(x_shard, np_dt):
    """[BPC,C,H,W] fp32 -> zero-padded [P, H+2, W+2] in np_dt."""
    xp = np.zeros((P, HP, WP), np_dt)
    xp[:, 1 : H + 1, 1 : W + 1] = x_shard.reshape(P, H, W)
    return xp


def _run(x, se, **spmd_kwargs):
    from concourse.bass_utils import run_bass_kernel_spmd

    nc = _get_prog()
    np_dt = np.float16 if COMPUTE == "f16" else np.float32
    x = np.asarray(x)
    se_p = np.tile(np.asarray(se, np.float32).reshape(C, K * K), (BPC, 1))
    in_maps = [
        {"x": _pad_shard(x[k * BPC : (k + 1) * BPC], np_dt), "se": se_p}
        for k in range(NCORES)
    ]
    res = run_bass_kernel_spmd(nc, in_maps, core_ids=list(range(NCORES)), **spmd_kwargs)
    out = np.empty((B, C, H, W), np.float32)
    for k in range(NCORES):
        out[k * BPC : (k + 1) * BPC] = (
            res.results[k]["o"].astype(np.float32).reshape(BPC, C, H, W)
        )
    return out, res


def kernel(x: np.ndarray, se: np.ndarray) -> np.ndarray:
    return _run(x, se)[0]


# revision 4
# speedup vs baseline: 26.7217x; 1.1757x over previous
"""Depthwise morphological (max-plus) dilation, 3x3, stride 1, zero-pad 1.

out[b,c,i,j] = max_{p,q} ( x_pad[b,c,i+p,j+q] + se[c,p,q] )

Sharding: pure data parallel over batch (16 batches -> 8 cores x 2).
On-core layout: partition dim = 2 batches x 64 channels = 128 planes;
each partition processes its own plane in row-blocks of R=32 output
rows. The host supplies x zero-padded to [P, H+2, W+2] and converted to
fp16, so all 9 taps are free-dim-shifted 3D views of one SBUF tile and
the device does no zero-fill.

Engine schedule (per output element: 9 scalar-adds + 8 tensor-maxes):
  DVE  tensor_scalar add runs in 4x perf mode   (~0.26 ns/elem/part)
  DVE  tensor_tensor max runs in 2x perf mode   (~0.52 ns/elem/part)
  ACT  activation-bias add, 1 elem/cycle @1.2GHz (~0.83 ns/elem/part)
GpSimd/Pool cannot run elementwise ops on this backend (ISA check
rejects TensorTensor on Pool), and the fused scalar_tensor_tensor loses
all DVE perf modes, so the optimum is unfused ops split DVE/ACT:
3 adds + 8 maxes on DVE (~4.95 ns/elem) and 6 adds on ACT (~5.0),
i.e. both engines balanced at the ~325 us/core compute floor
(memory roofline is ~95 us; 17 exact elementwise ops/elem bind first).
fp16 compute gives ~6e-4 max rel err vs the fp32 reference.
"""

import numpy as np

B, C, H, W = 16, 64, 256, 256
K = 3
NCORES = 8
BPC = B // NCORES          # batches per core
P = BPC * C                # 128 partitions
HP, WP = H + 2, W + 2      # host-padded plane

COMPUTE = "f16"            # "f16" (fast, ~6e-4 rel err) or "f32" (exact)
R = 32                     # output rows per block
DVE_TAPS = (0, 5, 8)       # tap t = di*3+dj; DVE does these adds (t0=root)
ACT_TAPS = (1, 2, 3, 4, 6, 7)  # adds on ScalarE; maxes on DVE

_prog_cache = {}


def _build(compute=COMPUTE, h=H, r=R, reps=1,
           dve_taps=DVE_TAPS, act_taps=ACT_TAPS,
           xbufs=2, abufs=2, atmp_bufs=6, dtmp_bufs=1):
    """Build the Bass program for one core: x [P,h+2,W+2] -> o [P,h,W]."""
    import concourse.bacc as bacc
    import concourse.mybir as mybir
    from concourse.tile import TileContext

    add, mx = mybir.AluOpType.add, mybir.AluOpType.max
    dt = mybir.dt.float16 if compute == "f16" else mybir.dt.float32

    nc = bacc.Bacc()
    x_d = nc.dram_tensor("x", [P, h + 2, W + 2], dt, kind="ExternalInput")
    se_d = nc.dram_tensor("se", [P, K * K], mybir.dt.float32, kind="ExternalInput")
    o_d = nc.dram_tensor("o", [P, h, W], dt, kind="ExternalOutput")

    assert len(dve_taps) + len(act_taps) == K * K

    with TileContext(nc) as tc:
        with (
            tc.tile_pool(name="cpool", bufs=1) as cpool,
            tc.tile_pool(name="xpool", bufs=xbufs) as xpool,
            tc.tile_pool(name="apool", bufs=abufs) as apool,
            tc.tile_pool(name="tpool", bufs=2) as tpool,
        ):
            se_sb = cpool.tile([P, K * K], mybir.dt.float32)
            nc.sync.dma_start(out=se_sb[:], in_=se_d[:, :])

            for r0 in [v for _ in range(reps) for v in range(0, h, r)]:
                xt = xpool.tile([P, r + 2, W + 2], dt, tag="xt")
                nc.sync.dma_start(out=xt[:], in_=x_d[:, r0 : r0 + r + 2, :])

                acc = apool.tile([P, r, W], dt, tag="acc")

                def src(t):
                    di, dj = divmod(t, K)
                    return xt[:, di : di + r, dj : dj + W]

                def sca(t):
                    return se_sb[:, t : t + 1]

                # All ACT adds issued at block start so ACT paces itself a
                # full block's worth of addends (6 bufs); the DVE chain then
                # runs its own links first and consumes the ACT tmps last.
                act_tmps = {}
                for t in act_taps:
                    tmp = tpool.tile([P, r, W], dt, tag="atmp", bufs=atmp_bufs)
                    nc.scalar.add(tmp[:], src(t), sca(t))
                    act_tmps[t] = tmp
                t0 = dve_taps[0]
                nc.vector.tensor_scalar(acc[:], src(t0), sca(t0), None, add)
                for t in dve_taps[1:]:
                    tmp = tpool.tile([P, r, W], dt, tag="dtmp", bufs=dtmp_bufs)
                    nc.vector.tensor_scalar(tmp[:], src(t), sca(t), None, add)
                    nc.vector.tensor_tensor(acc[:], acc[:], tmp[:], mx)
                for t in act_taps:
                    nc.vector.tensor_tensor(acc[:], acc[:], act_tmps[t][:], mx)

                nc.sync.dma_start(out=o_d[:, r0 : r0 + r, :], in_=acc[:])
    # bacc legalization (splits >1-wait instructions into event semaphores)
    nc.finalize()
    return nc


def _get_prog(key=("default",)):
    if key not in _prog_cache:
        _prog_cache[key] = _build()
    return _prog_cache[key]


def _pad_shard(x_shard, np_dt):
    """[BPC,C,H,W] fp32 -> zero-padded [P, H+2, W+2] in np_dt."""
    xp = np.zeros((P, HP, WP), np_dt)
    xp[:, 1 : H + 1, 1 : W + 1] = x_shard.reshape(P, H, W)
    return xp


def _run(x, se, **spmd_kwargs):
    from concourse.bass_utils import run_bass_kernel_spmd

    nc = _get_prog()
    np_dt = np.float16 if COMPUTE == "f16" else np.float32
    x = np.asarray(x)
    se_p = np.tile(np.asarray(se, np.float32).reshape(C, K * K), (BPC, 1))
    in_maps = [
        {"x": _pad_shard(x[k * BPC : (k + 1) * BPC], np_dt), "se": se_p}
        for k in range(NCORES)
    ]
    res = run_bass_kernel_spmd(nc, in_maps, core_ids=list(range(NCORES)), **spmd_kwargs)
    out = np.empty((B, C, H, W), np.float32)
    for k in range(NCORES):
        out[k * BPC : (k + 1) * BPC] = (
            res.results[k]["o"].astype(np.float32).reshape(BPC, C, H, W)
        )
    return out, res


def kernel(x: np.ndarray, se: np.ndarray) -> np.ndarray:
    return _run(x, se)[0]


# revision 6
# speedup vs baseline: 26.7536x; 1.0012x over previous
"""Depthwise morphological (max-plus) dilation, 3x3, stride 1, zero-pad 1.

out[b,c,i,j] = max_{p,q} ( x_pad[b,c,i+p,j+q] + se[c,p,q] )

Sharding: pure data parallel over batch (16 batches -> 8 cores x 2).
On-core layout: partition dim = 2 batches x 64 channels = 128 planes;
each partition processes its own plane in row-blocks of R=32 output
rows. The host supplies x zero-padded to [P, H+2, W+2] and converted to
fp16, so all 9 taps are free-dim-shifted 3D views of one SBUF tile and
the device does no zero-fill.

Engine schedule (per output element: 9 scalar-adds + 8 tensor-maxes):
  DVE  tensor_scalar add runs in 4x perf mode   (~0.26 ns/elem/part)
  DVE  tensor_tensor max runs in 2x perf mode   (~0.52 ns/elem/part)
  ACT  activation-bias add, 1 elem/cycle @1.2GHz (~0.83 ns/elem/part)
GpSimd/Pool cannot run elementwise ops on this backend (ISA check
rejects TensorTensor on Pool), and the fused scalar_tensor_tensor loses
all DVE perf modes, so the optimum is unfused ops split DVE/ACT:
3 adds + 8 maxes on DVE (~4.95 ns/elem) and 6 adds on ACT (~5.0),
i.e. both engines balanced at the ~325 us/core compute floor
(memory roofline is ~95 us; 17 exact elementwise ops/elem bind first).
fp16 compute gives ~6e-4 max rel err vs the fp32 reference.
"""

import numpy as np

B, C, H, W = 16, 64, 256, 256
K = 3
NCORES = 8
BPC = B // NCORES          # batches per core
P = BPC * C                # 128 partitions
HP, WP = H + 2, W + 2      # host-padded plane

COMPUTE = "f16"            # "f16" (fast, ~6e-4 rel err) or "f32" (exact)
R = 32                     # output rows per block
DVE_TAPS = (0, 5, 8)       # tap t = di*3+dj; DVE does these adds (t0=root)
ACT_TAPS = (1, 2, 3, 4, 6, 7)  # adds on ScalarE; maxes on DVE

_prog_cache = {}


def _build(compute=COMPUTE, h=H, r=R, reps=1,
           dve_taps=DVE_TAPS, act_taps=ACT_TAPS,
           xbufs=2, abufs=2, atmp_bufs=7, dtmp_bufs=1, dtmp_tag="dtmp"):
    """Build the Bass program for one core: x [P,h+2,W+2] -> o [P,h,W]."""
    import concourse.bacc as bacc
    import concourse.mybir as mybir
    from concourse.tile import TileContext

    add, mx = mybir.AluOpType.add, mybir.AluOpType.max
    dt = mybir.dt.float16 if compute == "f16" else mybir.dt.float32

    nc = bacc.Bacc()
    x_d = nc.dram_tensor("x", [P, h + 2, W + 2], dt, kind="ExternalInput")
    se_d = nc.dram_tensor("se", [P, K * K], mybir.dt.float32, kind="ExternalInput")
    o_d = nc.dram_tensor("o", [P, h, W], dt, kind="ExternalOutput")

    assert len(dve_taps) + len(act_taps) == K * K

    with TileContext(nc) as tc:
        with (
            tc.tile_pool(name="cpool", bufs=1) as cpool,
            tc.tile_pool(name="xpool", bufs=xbufs) as xpool,
            tc.tile_pool(name="apool", bufs=abufs) as apool,
            tc.tile_pool(name="tpool", bufs=2) as tpool,
        ):
            se_sb = cpool.tile([P, K * K], mybir.dt.float32)
            nc.sync.dma_start(out=se_sb[:], in_=se_d[:, :])

            for r0 in [v for _ in range(reps) for v in range(0, h, r)]:
                xt = xpool.tile([P, r + 2, W + 2], dt, tag="xt")
                nc.sync.dma_start(out=xt[:], in_=x_d[:, r0 : r0 + r + 2, :])

                acc = apool.tile([P, r, W], dt, tag="acc")

                def src(t):
                    di, dj = divmod(t, K)
                    return xt[:, di : di + r, dj : dj + W]

                def sca(t):
                    return se_sb[:, t : t + 1]

                # All ACT adds issued at block start so ACT paces itself a
                # full block's worth of addends (6 bufs); the DVE chain then
                # runs its own links first and consumes the ACT tmps last.
                act_tmps = {}
                for t in act_taps:
                    tmp = tpool.tile([P, r, W], dt, tag="atmp", bufs=atmp_bufs)
                    nc.scalar.add(tmp[:], src(t), sca(t))
                    act_tmps[t] = tmp
                t0 = dve_taps[0]
                nc.vector.tensor_scalar(acc[:], src(t0), sca(t0), None, add)
                for t in dve_taps[1:]:
                    tb = atmp_bufs if dtmp_tag == "atmp" else dtmp_bufs
                    tmp = tpool.tile([P, r, W], dt, tag=dtmp_tag, bufs=tb)
                    nc.vector.tensor_scalar(tmp[:], src(t), sca(t), None, add)
                    nc.vector.tensor_tensor(acc[:], acc[:], tmp[:], mx)
                for t in act_taps:
                    nc.vector.tensor_tensor(acc[:], acc[:], act_tmps[t][:], mx)

                nc.sync.dma_start(out=o_d[:, r0 : r0 + r, :], in_=acc[:])
    # bacc legalization (splits >1-wait instructions into event semaphores)
    nc.finalize()
    return nc


def _get_prog(key=("default",)):
    if key not in _prog_cache:
        _prog_cache[key] = _build()
    return _prog_cache[key]


def _pad_shard(x_shard, np_dt):
    """[BPC,C,H,W] fp32 -> zero-padded [P, H+2, W+2] in np_dt."""
    xp = np.zeros((P, HP, WP), np_dt)
    xp[:, 1 : H + 1, 1 : W + 1] = x_shard.reshape(P, H, W)
    return xp


def _run(x, se, **spmd_kwargs):
    from concourse.bass_utils import run_bass_kernel_spmd

    nc = _get_prog()
    np_dt = np.float16 if COMPUTE == "f16" else np.float32
    x = np.asarray(x)
    se_p = np.tile(np.asarray(se, np.float32).reshape(C, K * K), (BPC, 1))
    in_maps = [
        {"x": _pad_shard(x[k * BPC : (k + 1) * BPC], np_dt), "se": se_p}
        for k in range(NCORES)
    ]
    res = run_bass_kernel_spmd(nc, in_maps, core_ids=list(range(NCORES)), **spmd_kwargs)
    out = np.empty((B, C, H, W), np.float32)
    for k in range(NCORES):
        out[k * BPC : (k + 1) * BPC] = (
            res.results[k]["o"].astype(np.float32).reshape(BPC, C, H, W)
        )
    return out, res


def kernel(x: np.ndarray, se: np.ndarray) -> np.ndarray:
    return _run(x, se)[0]
